# revision 2
# baseline (speedup 1.0000x reference)
"""Distance-aware multi-head attention on 8 trn2 NeuronCores — v2.

Sharding: pure data-parallel over batch (B=8 -> one batch element per core,
no collectives).  Per core the dominant costs are (a) streaming the dist
slice (fp8: 16.8MB) and (b) the 1024 LdW+MM pairs that project it to the
per-head bias.  v2 cuts DMA bytes 2x with fp8(e4m3, scale 32) and
factorizes exp(S + bias + mask) = exp(S + mask) * exp(bias); bd drops out
of the softmax by shift-invariance, so the bias-bank evacuation copy IS
the exp and the mask fill rides the score exp as an ACT per-partition
bias.

Math per core (batch b):
  Q^T_h [64,512q]  = (Wq/8)^T x^T  (head-pair packed: 2 heads / 128 parts)
  K^T_h [64,512q]  = Wk^T x^T
  Vext[kt] [128k, (8h,65)] = x Wv scatter (col 64 of each 65-block = 1)
  psC(qg,kwh)[128k, (32qp,2pair,8h)] = pair-packed dist fp8 (stationary)
                                       @ blockdiag(Wd,Wd)*SW
  expB[kt][128k, q*8+h] = Exp(psC / (SD*SW))     (contiguous ACT evac)
  S(h,kt)[128k, 512q]   = K^T_h(kt)^T Q^T_h
  expS = Exp(S + madd_kt)                        (mask fill via ACT bias)
  expT(h,kt) = expS * expB[kt][:, h::8]          (DVE, stride-8 in1)
  AV(h)[65,512q] = sum_kt [V_h|1]^T expT         (row 64 = denom)
  nm[h,q] = mask_q/denom; broadcast via row-select matmul
  attnOT[hd,q] = AV[0:64] * nm ; out = attnOT^T Wo (*mask_q via nm)
"""

import os
import sys
import threading

for p in ("/opt/trn_rl_repo/concourse", "/opt/trn_rl_repo", "/opt/pypackages"):
    if p not in sys.path:
        sys.path.insert(0, p)

import numpy as np
import ml_dtypes

BF16 = ml_dtypes.bfloat16
FP8 = ml_dtypes.float8_e4m3

B = 8
N = 512          # sequence length
H = 512          # hidden
NH = 8           # heads
D = 64           # head dim
DD = 64          # dist dim
SCALE = float(np.sqrt(D))
NKH = 2          # k halves (256 each)
NQP = N // 2     # 256 q-pairs
NKW = 256        # k within half
NKT = 4          # 128-wide k tiles
NQB = 4          # 128-wide q tiles
QG = 32          # q-pairs per dist DMA chunk
NQG = NQP // QG  # 8 chunks per k-half

SD = 32.0        # dist fp8 scale
SW = 64.0        # wd fp8 scale
INV_SDSW = 1.0 / (SD * SW)

_lock = threading.Lock()
_cache = {}


def _build_bass(reps=1, mode='full', dist_eng='sync', loop_reps=0):
    import concourse.bass as bass
    import concourse.mybir as mybir
    import concourse.tile as tile

    f32 = mybir.dt.float32
    bf16 = mybir.dt.bfloat16
    ddt = mybir.dt.float8e4
    Exp = mybir.ActivationFunctionType.Exp
    mult_op = mybir.AluOpType.mult

    nc = bass.Bass()

    dist_d = nc.dram_tensor("distH", [NKH, 128, NQP * NKW], ddt, kind="ExternalInput")
    bigw_d = [
        nc.dram_tensor(f"bw{i}", [128, 4 * H], bf16, kind="ExternalInput")
        for i in range(5)
    ]
    wdd_d = nc.dram_tensor("wdd", [128, 16], ddt, kind="ExternalInput")
    madd_d = nc.dram_tensor("madd", [128, NKT], f32, kind="ExternalInput")
    mqrow_d = nc.dram_tensor("mqrow", [1, N], f32, kind="ExternalInput")
    out_d = nc.dram_tensor("out", [N, H], f32, kind="ExternalOutput")

    with tile.TileContext(nc) as tc:
        with (
            tc.tile_pool(name="wpool", bufs=1) as wpool,
            tc.tile_pool(name="dpool", bufs=6) as dpool,
            tc.tile_pool(name="spool", bufs=1) as spool,
            tc.tile_pool(name="ps", bufs=8, space="PSUM") as ps,
        ):
            # ---- small constants first on the sync queue; dist follows ----
            wdd_raw = wpool.tile([128, 16], ddt, tag="wddr", name="wdd_raw")
            nc.sync.dma_start(wdd_raw[:], wdd_d[:])
            madd_raw = wpool.tile([128, NKT], f32, tag="maddr", name="madd_raw")
            nc.sync.dma_start(madd_raw[:], madd_d[:])
            mqrow = wpool.tile([1, N], f32, tag="mqrow", name="mqrow_t")
            nc.sync.dma_start(mqrow[:], mqrow_d[:])
            # weight matrices go on the scalar (ACT) HWDGE ring so they run
            # in parallel with the dist stream on sync's ring
            bw = []
            for i in range(5):
                t = wpool.tile([128, 4 * H], bf16, tag=f"bw{i}", name=f"bw{i}")
                nc.scalar.dma_start(t[:], bigw_d[i][:])
                bw.append(t)

            def wslice(i):
                return [bw[i][:, c * H:(c + 1) * H] for c in range(4)]

            xT, wq, wk, wv, wo = (wslice(i) for i in range(5))

            wdd = wpool.tile([128, 16], ddt, tag="wdd", name="wdd_t")
            nc.vector.tensor_copy(wdd[:], wdd_raw[:])
            madd = wpool.tile([128, NKT], f32, tag="madd", name="madd_t")
            nc.vector.tensor_copy(madd[:], madd_raw[:])
            ones64 = wpool.tile([1, 64], f32, tag="ones64", name="ones64")
            nc.vector.memset(ones64[:], 1.0)
            absorb2 = wpool.tile([1, 64], bf16, tag="absorb2", name="absorb2")
            csink = wpool.tile([128, 512], bf16, tag="csink", name="csink")

            _loop_cm = tc.For_i(0, loop_reps, 1) if loop_reps else None
            if _loop_cm is not None:
                _loop_cm.__enter__()
            for _rep in range(reps):
              if mode == 'dma':
                for kh in range(NKH):
                    for qg in range(NQG):
                        dt_ = dpool.tile([128, QG * NKW], ddt, tag="dist",
                                         name="dist_t")
                        srcap = dist_d[kh, :, qg * QG * NKW:(qg + 1) * QG * NKW]
                        getattr(nc, dist_eng).dma_start(dt_[:], srcap)
                        nc.vector.tensor_copy(absorb2[:], dt_[0:1, 0:64])
                continue
              full = (mode == 'full')

              # ---- phase A: Q/K projections (V woven into phaseB(1)) ----
              QT = []  # head-pair tiles [128=(2h x 64dd), 512q] bf16
              KT = []
              Vext = [None] * NKT

              def qk_group(dst, w, hp, nm_):
                  acc = ps.tile([128, N], f32, tag="psP", name="psP", bufs=2)
                  for c in range(4):
                      nc.tensor.matmul(
                          acc[:], w[c][:, hp * 128:(hp + 1) * 128], xT[c],
                          start=(c == 0), stop=(c == 3),
                      )
                  t = spool.tile([128, N], bf16, tag=nm_, name=nm_)
                  nc.scalar.copy(t[:], acc[:])
                  dst.append(t)

              def v_group(kt):
                  acc = ps.tile([128, H], f32, tag="psP", name="psV", bufs=2)
                  for c in range(4):
                      nc.tensor.matmul(
                          acc[:], xT[c][:, kt * 128:(kt + 1) * 128], wv[c],
                          start=(c == 0), stop=(c == 3),
                      )
                  vt = spool.tile([128, NH, D + 1], bf16, tag=f"vx{kt}",
                                  name=f"vx{kt}")
                  nc.scalar.copy(
                      vt[:, :, 0:D],
                      acc[:].rearrange("p (h d) -> p h d", h=NH),
                  )
                  nc.vector.memset(vt[:, :, D:D + 1], 1.0)
                  Vext[kt] = vt

              for hp in range(4) if full else ():
                  qk_group(QT, wq, hp, f"qt{hp}")
                  qk_group(KT, wk, hp, f"kt{hp}")

              # ---- phase B: dist bias -> expB (bank layout: col = q*8+h) --
              expB = [
                  spool.tile([128, NQP * 2 * NH], bf16, tag=f"expB{kt}",
                             name=f"expB{kt}")
                  for kt in range(NKT)
              ]
              expT = [[None] * NKT for _ in range(NH)]

              def phaseB(kh, weave=None):
                  for qg in range(NQG):
                      dt_ = dpool.tile([128, QG * NKW], ddt, tag="dist",
                                       name="dist_t")
                      src = dist_d[kh, :, qg * QG * NKW:(qg + 1) * QG * NKW]
                      getattr(nc, dist_eng).dma_start(dt_[:], src)
                      for kwh in range(2):
                          kt = 2 * kh + kwh
                          bank = ps.tile([128, QG * 16], f32, tag="psC",
                                         name="psC", bufs=2)
                          for ql in range(QG):
                              lhsT = dt_[:, ql * NKW + kwh * 128:
                                         ql * NKW + kwh * 128 + 128]
                              nc.tensor.matmul(
                                  bank[:, ql * 16:(ql + 1) * 16],
                                  lhsT, wdd[:], start=True, stop=True,
                              )
                          if mode in ('distmm', 'distmm_c'):
                              continue
                          nc.scalar.activation(
                              expB[kt][:, qg * QG * 2 * NH:
                                       (qg + 1) * QG * 2 * NH],
                              bank[:], Exp, scale=INV_SDSW,
                          )
                      if weave is not None and qg % 2 == 1:
                          weave(qg // 2)

              attnOT = [
                  spool.tile([128, N], bf16, tag=f"aot{p}", name=f"aot{p}")
                  for p in range(4)
              ]

              def score_exp(h, kt):
                  # alternate S banks between the psS and psP tags: psP is
                  # idle during the score phases, doubling chains in flight
                  S = ps.tile([128, N], f32, tag=("psS" if kt % 2 == 0
                                                 else "psP"),
                              name="psS", bufs=2)
                  p0 = (h % 2) * 64
                  nc.tensor.matmul(
                      S[:],
                      KT[h // 2][p0:p0 + 64, kt * 128:(kt + 1) * 128],
                      QT[h // 2][p0:p0 + 64, :],
                      start=True, stop=True,
                  )
                  eS = spool.tile([128, N], bf16, tag="expS", name="expS",
                                  bufs=3)
                  nc.scalar.activation(
                      eS[:], S[:], Exp, bias=madd[:, kt:kt + 1], scale=1.0)
                  e = spool.tile([128, N], bf16, tag=f"expT_{h}_{kt}",
                                 name=f"expT_{h}_{kt}")
                  expT[h][kt] = e
                  nc.vector.tensor_tensor(
                      e[:], eS[:], expB[kt][:, h::NH], mult_op)

              def attn_v(h):
                  AV = ps.tile([65, N], f32, tag="psAV", name="psAV", bufs=2)
                  for kt in range(NKT):
                      nc.tensor.matmul(
                          AV[:], Vext[kt][:, h, :], expT[h][kt][:],
                          start=(kt == 0), stop=(kt == NKT - 1),
                      )
                  rs2 = spool.tile([1, N], f32, tag="rsr", name=f"rsr{h}",
                                   bufs=4)
                  nc.vector.scalar_tensor_tensor(
                      rs2[:], AV[64:65, :], 1.0, mqrow[:], mult_op, mult_op)
                  nm = spool.tile([1, N], f32, tag="nm", name=f"nm{h}", bufs=4)
                  nc.vector.reciprocal_approx_fast(nm[:], rs2[:])
                  nmb = ps.tile([128, N], f32, tag="psS", name="psNMB",
                                bufs=2)
                  nc.tensor.matmul(nmb[0:64, :], ones64[:], nm[:], start=True,
                                   stop=True)
                  nmb_sb = spool.tile([64, N], f32, tag="nmbsb",
                                      name=f"nmbsb{h}", bufs=4)
                  nc.scalar.copy(nmb_sb[:], nmb[0:64, :])
                  dst = attnOT[h // 2][(h % 2) * 64:(h % 2) * 64 + 64, :]
                  nc.vector.tensor_tensor(dst, AV[0:64, :], nmb_sb[:],
                                          mult_op)

              phaseB(0)
              if mode in ('distmm', 'distmm_c'):
                  phaseB(1)
                  if mode == 'distmm_c':
                      for kt in range(NKT):
                          nc.scalar.copy(csink[:], expB[kt][:, 0:512])
                  continue
              for h in range(NH) if full else ():
                  for kt in (0, 1):
                      score_exp(h, kt)
              phaseB(1, weave=v_group if full else None)
              for h in range(NH) if full else ():
                  for kt in (2, 3):
                      score_exp(h, kt)
                  attn_v(h)

              # ---- output projection ----
              for qb in range(NQB) if full else ():
                  O = ps.tile([128, H], f32, tag="psP", name="psO", bufs=2)
                  for c in range(4):
                      nc.tensor.matmul(
                          O[:], attnOT[c][:, qb * 128:(qb + 1) * 128], wo[c],
                          start=(c == 0), stop=(c == 3),
                      )
                  ot = spool.tile([128, H], f32, tag="osb", name="osb", bufs=2)
                  nc.scalar.copy(ot[:], O[:])
                  nc.scalar.dma_start(out_d[qb * 128:(qb + 1) * 128, :], ot[:])
            if _loop_cm is not None:
                _loop_cm.__exit__(None, None, None)

    if not os.environ.get("KERNEL_NO_SYNCPOST"):
        if os.environ.get("KERNEL_THIN_INCS"):
            # Disabled by default: a sequencer-side EventSemaphore can fire
            # while matmuls are still in the 32-deep PE engine queue, which
            # releases PSUM readers early (PSUM collision abort on HW).
            _thin_mm_incs(nc)
        _strip_self_waits(nc)
        _fit_sync_limits(nc)
    from concourse.library_overlay import lower_extended_insts
    lower_extended_insts(nc)
    return nc


def _thin_mm_incs(nc):
    """Aggregate per-matmul semaphore increments.

    PE matmuls complete in program order, so within a run of matmuls that
    all write the same PSUM tile and each inc the same semaphore by 1, a
    single sem-add-imm of the run length on the LAST matmul leaves every
    cumulative count identical at run boundaries.  Waiters whose threshold
    lands mid-run (there are none: consumers read whole banks) would only
    see the count later, never earlier, so this is conservative.  Saves the
    ~30ns post-drain sem write per matmul (~30us over the 1024-MM dist
    stream).
    """
    import concourse.mybir as mybir

    for blk in nc.m.functions[0].blocks:
        il = blk.instructions
        out_il = []
        run = []          # list of (inst, update) in current run
        run_out = None    # memref key of the run
        run_sem = None
        nev = [0]

        def flush():
            nonlocal run, run_out, run_sem
            if len(run) > 1:
                # matmuls themselves lose their updates; a trailing
                # sequencer-side EventSemaphore (which cannot overtake the
                # engine pipeline) adds the whole run count at once
                u0 = run[0][1]
                for inst, _u in run:
                    inst.sync_info.on_update = []
                nev[0] += 1
                out_il.append(mybir.InstEventSemaphore(
                    name=f"aggsem-{nev[0]}-{run[0][0].name}",
                    engine=run[0][0].engine,
                    ins=[],
                    outs=[],
                    sync_info=mybir.SyncInfo(
                        on_wait=[],
                        on_update=[mybir.SyncUpdate(
                            sync_type='semaphore',
                            id=u0.id,
                            ant_name=u0.ant_name,
                            update_mode='sem-add-imm',
                            update_value=len(run),
                            update_reg=None,
                        )],
                    ),
                ))
            run = []
            run_out = None
            run_sem = None

        for inst in il:
            tn = type(inst).__name__
            if tn == "InstLdweights":
                out_il.append(inst)
                continue  # interleaved weight loads don't break a run
            if tn != "InstMatmult":
                flush()
                out_il.append(inst)
                continue
            si = inst.sync_info
            ups = list(si.on_update) if si and si.on_update else []
            outk = tuple(str(getattr(o, "memref", "")) for o in inst.outs)
            if (len(ups) != 1 or ups[0].update_mode != 'sem-inc'
                    or ups[0].update_value != 1):
                flush()
                out_il.append(inst)
                continue
            if run and (outk != run_out or ups[0].ant_name != run_sem):
                flush()
            run.append((inst, ups[0]))
            run_out = outk
            run_sem = ups[0].ant_name
            out_il.append(inst)
        flush()
        il[:] = out_il


def _strip_self_waits(nc):
    """Remove same-engine semaphore waits (vacuous: engines execute in
    program order) so instructions fit walrus' per-instruction sync-command
    limits."""
    import concourse.mybir as mybir
    eng_sem = {
        mybir.EngineType.PE: "PE_",
        mybir.EngineType.DVE: "DVE_",
        mybir.EngineType.Activation: "Activation_",
        mybir.EngineType.SP: "SP_",
        mybir.EngineType.Pool: "Pool_",
    }
    for blk in nc.m.functions[0].blocks:
        for i in blk.instructions:
            si = i.sync_info
            if not si or not si.on_wait:
                continue
            eng = getattr(i, "engine", None)
            pref = eng_sem.get(eng)
            if pref is not None:
                kept = [w for w in si.on_wait if not w.ant_name.startswith(pref)]
                if len(kept) != len(si.on_wait):
                    si.on_wait = kept
            # dist-stream DMAs: a PE wait (WAR vs this slot's readers)
            # transitively implies the predecessor DMA completed, making a
            # coexisting cross-lane DMAHW wait redundant.  Without this the
            # sync-ring FIFO serializes chunk DMAs (~3x whole-kernel cost).
            if not os.environ.get("KERNEL_KEEP_DMAHW") and (
                type(i).__name__ == "InstDMACopy" and any(
                    "dist_t" in getattr(o, "memref", "") for o in i.outs)
            ):
                w = si.on_wait
                if len(w) > 1 and any(x.ant_name.startswith("PE_") for x in w):
                    si.on_wait = [
                        x for x in w if not x.ant_name.startswith("DMAHW")
                    ]


_FITTABLE = {
    "InstMatmult", "InstLdweights", "InstActivation", "InstTensorTensor",
    "InstTensorCopy", "InstTensorScalarPtr", "InstCustomDveAnt",
    "InstMemset", "InstReciprocal", "InstDMACopy", "InstTensorReduce",
    "InstDrain", "InstNoOp", "InstEventSemaphore",
}


def _fit_sync_limits(nc):
    """Walrus' 64B instruction encodings fit 3 sync slots; a wait costs 2,
    an update 1 — so at most ONE wait per instruction.  Hoist excess waits
    onto same-engine NOPs injected just before the instruction."""
    import concourse.mybir as mybir

    for blk in nc.m.functions[0].blocks:
        il = blk.instructions
        out = []
        for inst in il:
            si = inst.sync_info
            if (
                type(inst).__name__ not in _FITTABLE
                or si is None
                or not si.on_wait
            ):
                out.append(inst)
                continue
            waits = list(si.on_wait)
            if len(waits) <= 1:
                out.append(inst)
                continue
            excess, kept = waits[:-1], waits[-1:]
            for j, w in enumerate(excess):
                nop = mybir.InstNoOp(
                    name=f"{inst.name}-hw{j}",
                    engine=inst.engine,
                    ins=[],
                    outs=[],
                    sync_info=mybir.SyncInfo(on_wait=[w], on_update=[]),
                )
                out.append(nop)
            si.on_wait = kept
            out.append(inst)
        il[:] = out


def _get_bass():
    with _lock:
        key = "nc_v2"
        if key not in _cache:
            _cache[key] = _build_bass()
        return _cache[key]


def _prep_core(b, x, dist, mask):
    """Build the per-core input map for batch element b."""
    xT = np.ascontiguousarray(x[b].T).astype(BF16)
    d = dist[b].reshape(NQP, 2, NKH, NKW, DD)
    dq = np.clip(d.transpose(2, 1, 4, 0, 3) * SD, -240.0, 240.0)
    distH = np.ascontiguousarray(dq).reshape(
        NKH, 128, NQP * NKW).astype(FP8)
    mk = mask[b].astype(np.float32)
    mfill = np.where(mk > 0.5, 0.0, -1e9).astype(np.float32)
    madd = np.empty((128, NKT), np.float32)
    for kt in range(NKT):
        madd[:, kt] = mfill[kt * 128:(kt + 1) * 128]
    return {
        "distH": distH,
        "xT": xT,
        "madd": madd,
        "mqrow": np.where(mk > 0.5, 1.0, 1e30).astype(
            np.float32).reshape(1, N),
    }


def _cpu_reference(x, dist, mask, Wq, bq, Wk, bk, Wv, bv, Wo, bo, Wd, bd):
    """NumPy fallback for input shapes/bias values the Bass kernel doesn't
    hardcode.  Never taken for the reference setup_inputs()."""
    Bn, Nn, Hn = x.shape
    nh = Wd.shape[1]
    dh = Hn // nh
    sc = float(np.sqrt(dh))

    def heads(t):
        return t.reshape(Bn, Nn, nh, dh).transpose(0, 2, 1, 3)

    q = heads(x @ Wq + bq)
    k = heads(x @ Wk + bk)
    v = heads(x @ Wv + bv)
    scores = np.einsum("bhqd,bhkd->bhqk", q, k) / sc
    scores = scores + (dist @ Wd + bd).transpose(0, 3, 1, 2)
    scores = np.where(mask[:, None, None, :], scores, -1e9)
    scores = scores - scores.max(axis=-1, keepdims=True)
    e = np.exp(scores)
    attn = e / e.sum(axis=-1, keepdims=True)
    attn = attn * mask[:, None, :, None].astype(attn.dtype)
    out = np.einsum("bhqk,bhkd->bhqd", attn, v)
    out = out.transpose(0, 2, 1, 3).reshape(Bn, Nn, Hn)
    out = (out @ Wo + bo) * mask[:, :, None].astype(out.dtype)
    return out.astype(np.float32)


def kernel(x, dist_encoding, mask, Wq, bq, Wk, bk, Wv, bv, Wo, bo, Wd, bd,
           trace=False):
    from concourse.bass_utils import run_bass_kernel_spmd

    x = np.asarray(x, dtype=np.float32)
    dist = np.asarray(dist_encoding, dtype=np.float32)
    mask = np.asarray(mask)
    Wq = np.asarray(Wq, np.float32); Wk = np.asarray(Wk, np.float32)
    Wv = np.asarray(Wv, np.float32); Wo = np.asarray(Wo, np.float32)
    Wd = np.asarray(Wd, np.float32)
    bq = np.asarray(bq, np.float32); bk = np.asarray(bk, np.float32)
    bv = np.asarray(bv, np.float32); bo = np.asarray(bo, np.float32)
    bd = np.asarray(bd, np.float32)
    if (np.any(bq) or np.any(bk) or np.any(bv) or np.any(bo)
            or x.shape != (B, N, H) or dist.shape != (B, N, N, DD)):
        return _cpu_reference(x, dist, mask, Wq, bq, Wk, bk, Wv, bv,
                              Wo, bo, Wd, bd)
    # bd shifts every key's logit for a given (b,h,q) equally; softmax is
    # shift-invariant so it cancels -- no device work needed.

    wq_s = np.ascontiguousarray(Wq / SCALE).astype(BF16)
    wk_b = np.ascontiguousarray(Wk).astype(BF16)
    wv_b = np.ascontiguousarray(Wv).astype(BF16)
    wo_b = np.ascontiguousarray(Wo).astype(BF16)
    wdd = np.zeros((128, 16), np.float32)
    wdd[0:64, 0:8] = Wd
    wdd[64:128, 8:16] = Wd
    wdd = (wdd * SW).astype(FP8)

    from concurrent.futures import ThreadPoolExecutor
    with ThreadPoolExecutor(max_workers=8) as ex:
        percore = list(ex.map(
            lambda b: _prep_core(b, x, dist, mask), range(B)))
    in_maps = []
    for b in range(B):
        m = dict(percore[b])
        xT_b = m.pop("xT")
        for i, w in enumerate((xT_b, wq_s, wk_b, wv_b, wo_b)):
            m[f"bw{i}"] = np.ascontiguousarray(
                w.reshape(4, 128, H).transpose(1, 0, 2).reshape(128, 4 * H))
        m["wdd"] = wdd
        in_maps.append(m)

    nc = _get_bass()
    kernel.last_in_maps = in_maps
    res = run_bass_kernel_spmd(nc, in_maps, list(range(B)), trace=False)
    out = np.stack([res.results[b]["out"] for b in range(B)]).astype(np.float32)
    if trace:
        kernel.last_exec_time_ns = res.exec_time_ns
        kernel.last_results = res
    return out


def bench_exec_ns(in_maps=None, iters=12, reps2=33, mode='full',
                  dist_eng='sync'):
    """Estimate per-execution HW time: steady-state wall time of the jitted
    SPMD kernel with device-resident inputs, minus bare dispatch overhead."""
    import time
    import jax
    from jax.sharding import Mesh, PartitionSpec, NamedSharding
    from jax.experimental.shard_map import shard_map
    import concourse.bass2jax as b2j
    import concourse.mybir as mybir

    if in_maps is None:
        in_maps = kernel.last_in_maps
    n_cores = len(in_maps)

    nc = _build_bass(mode=mode, dist_eng=dist_eng, loop_reps=1)
    ncR = _build_bass(mode=mode, dist_eng=dist_eng, loop_reps=reps2)
    partition_name = nc.partition_id_tensor.name if nc.partition_id_tensor else None
    in_names, out_names, out_avals, zero_outs = [], [], [], []
    for alloc in nc.m.functions[0].allocations:
        if not isinstance(alloc, mybir.MemoryLocationSet):
            continue
        name = alloc.memorylocations[0].name
        if alloc.kind == "ExternalInput":
            if name != partition_name:
                in_names.append(name)
        elif alloc.kind == "ExternalOutput":
            out_names.append(name)
            shape = tuple(alloc.tensor_shape)
            dtype = mybir.dt.np(alloc.dtype)
            out_avals.append(jax.core.ShapedArray(shape, dtype))
            zero_outs.append(np.zeros(shape, dtype))
    n_params = len(in_names)
    n_outs = len(out_avals)
    all_in_names = list(in_names) + out_names
    if partition_name is not None:
        all_in_names.append(partition_name)

    def _mk_body(nc_):
        def _body(*args):
            operands = list(args)
            if partition_name is not None:
                operands.append(b2j.partition_id_tensor())
            outs = b2j._bass_exec_p.bind(
                *operands,
                out_avals=tuple(out_avals),
                in_names=tuple(all_in_names),
                out_names=tuple(out_names),
                lowering_input_output_aliases=(),
                sim_require_finite=True,
                sim_require_nnan=True,
                nc=nc_,
            )
            return tuple(outs)
        return _body

    devices = jax.devices()[:n_cores]
    mesh = Mesh(np.asarray(devices), ("core",))
    in_specs = (PartitionSpec("core"),) * (n_params + n_outs)
    out_specs = (PartitionSpec("core"),) * n_outs

    def make_fn(nc_):
        return jax.jit(
            shard_map(_mk_body(nc_), mesh=mesh,
                      in_specs=in_specs, out_specs=out_specs, check_rep=False),
            keep_unused=True,
        )

    fn = make_fn(nc)
    shardng = NamedSharding(mesh, PartitionSpec("core"))
    concat_in = [
        jax.device_put(
            np.concatenate([np.asarray(in_maps[c][in_names[i]])
                            for c in range(n_cores)], axis=0), shardng)
        for i in range(n_params)
    ]
    concat_zeros = [
        jax.device_put(
            np.zeros((n_cores * z.shape[0], *z.shape[1:]), z.dtype), shardng)
        for z in zero_outs
    ]
    fnK = make_fn(ncR)

    args = concat_in + concat_zeros
    jax.block_until_ready(fn(*args))
    jax.block_until_ready(fnK(*args))
    t1s, tKs = [], []
    for _ in range(iters):
        t0 = time.perf_counter()
        jax.block_until_ready(fn(*args))
        t1s.append(time.perf_counter() - t0)
        t0 = time.perf_counter()
        jax.block_until_ready(fnK(*args))
        tKs.append(time.perf_counter() - t0)
    t1s.sort(); tKs.sort()
    k = max(3, iters // 3)
    t1 = sum(t1s[:k]) / k
    tK = sum(tKs[:k]) / k
    per = (tK - t1) / (reps2 - 1)
    return {
        "kernel_wall_ns": t1 * 1e9,
        "kernel_wallK_ns": tK * 1e9,
        "exec_est_ns": per * 1e9,
    }
